# revision 2
# baseline (speedup 1.0000x reference)
"""Trainium2 Bass kernel for BinarizeConv2dSDP.

Reference math (forward only):
    w    = rsqrt(m^2 + sum_k z_k^2/100) * (m + rv @ z)   elementwise
    bw   = sign(w)        -- the positive rsqrt factor drops out of sign()
    ba   = sign(x)
    out  = conv2d(ba, bw, pad=1, NCHW/OIHW) * alpha[o]

Sharding (8 cores): weight prep is sharded 8-way over out-channels (each
core reads only its 32-o slice of Z/M = 2.65 MB), the 74 KB of binarized
fp8 conv weights are AllGathered across the chip, and the conv runs
batch-8 data-parallel (8 images x all 256 out-channels per core). This
cuts per-core HBM traffic from ~30 MB (batch-4 x o-2 grid, weights 4x
replicated) to ~16 MB, which moves the kernel off the DMA roofline; the
conv phase itself runs at the fp8 DoubleRow matmul roofline.

Per-core pipeline:
  - Z slice lands as [128 part(a*32+oo), 2 b, 2304] where k = b*4+a; DVE
    folds the b-pairs with per-partition rv columns (rvA/rvB built by two
    tiny selection matmuls from an [8,1] rv column).
  - a-group reduction + transpose + M-add fused on the PE: per (cc,tap),
    matmul(t_block[128, c_low] x S_sel[128->32]) + matmul(M_block[32,
    c_low] x eye32), accumulating [c_low, 32 o] directly in PSUM in conv
    layout; ACT signs 128-col groups into the fp8 AG input tile.
  - AllGather (ncfw collective, ~5 us) of [128, 2cc*9t*32o] fp8 slices;
    8 strided DMAs scatter ranks into wt [128 c_low, 2 cc, 9 t, 256 o].
  - activations: per image [128 part(c_low), 2 c-chunk, 912] fp8 zero-
    padded 30x30 images; border memsets on DVE (dep-free, run during the
    Z window); 2 images binarized on DVE to +-0.5 (drains use 2*alpha),
    6 signed on ACT behind the weight signs.
  - conv: per image, 2 o-halves x 2 half-image PSUM groups; each
    (tap, o-half) DoubleRow LDWEIGHTS is shared by the two half-image
    matmuls; psum drains (x alpha) on DVE; outputs ride the ACT HWDGE
    ring while inputs keep the SP ring.
"""

import sys

for _p in ("/opt/trn_rl_repo",):
    if _p not in sys.path:
        sys.path.insert(0, _p)

import contextlib

import numpy as np

import concourse.bass as bass
import concourse.bacc as bacc
import concourse.tile as tile
from concourse import mybir
from concourse.bass_utils import run_bass_kernel_spmd
from concourse.tile_rust import add_dep_helper

N_CORES = 8
B = 64
B_SH = 8        # images per core (batch/8)
C = 256         # in channels
O = 256
O_LOC = 32      # out channels computed per core (o/8)
K = 8           # SDP rank
KK = 9          # 3x3 taps
CT = C * KK     # 2304
HCT = CT // 2   # 1152 (one c-chunk)
H = 28
HP = 30         # padded row width
PADW = 912      # 30*30=900 padded to %16
WCOLS = 2 * KK * O_LOC  # 576 fp8 cols per core in the AG payload
F32 = mybir.dt.float32
FP8 = mybir.dt.float8e4


def _sel_consts():
    selA = np.zeros((K, 128), dtype=np.float32)
    selB = np.zeros((K, 128), dtype=np.float32)
    for p in range(128):
        selA[p // 32, p] = 1.0
        selB[4 + p // 32, p] = 1.0
    s_sel = np.zeros((128, O_LOC), dtype=np.float32)
    for p in range(128):
        s_sel[p, p % O_LOC] = 1.0
    eye32 = np.eye(O_LOC, dtype=np.float32)
    return selA, selB, s_sel, eye32


def _build_kernel(tc, x_t, z_t, m_t, a_t, rv_t, selA_t, selB_t, ssel_t, eye_t,
                  ag_in, ag_out, out_t):
    nc = tc.nc
    ctx = contextlib.ExitStack()
    consts = ctx.enter_context(tc.tile_pool(name="consts", bufs=1))
    zpool = ctx.enter_context(tc.tile_pool(name="zpool", bufs=1))
    wpool = ctx.enter_context(tc.tile_pool(name="wpool", bufs=1))
    stage = ctx.enter_context(tc.tile_pool(name="stage", bufs=8))
    acts = ctx.enter_context(tc.tile_pool(name="acts", bufs=1))
    outp = ctx.enter_context(tc.tile_pool(name="outp", bufs=8))
    psums = ctx.enter_context(tc.tile_pool(name="psums", bufs=6, space="PSUM"))
    pst = ctx.enter_context(tc.tile_pool(name="pst", bufs=2, space="PSUM"))

    with ctx:
        # ---- tiny constants on the gpsimd HWDGE ring ----
        rv8 = consts.tile([K, 1], F32, name="rv8")
        nc.gpsimd.dma_start(rv8, rv_t.ap().rearrange("a k -> k a"))
        selA_sb = consts.tile([K, 128], F32, name="selA_sb")
        nc.gpsimd.dma_start(selA_sb, selA_t.ap())
        selB_sb = consts.tile([K, 128], F32, name="selB_sb")
        nc.gpsimd.dma_start(selB_sb, selB_t.ap())
        ssel_sb = consts.tile([128, O_LOC], F32, name="ssel_sb")
        nc.gpsimd.dma_start(ssel_sb, ssel_t.ap())
        eye_sb = consts.tile([O_LOC, O_LOC], F32, name="eye_sb")
        nc.gpsimd.dma_start(eye_sb, eye_t.ap())
        alpha_sb = consts.tile([128, 2], F32, name="alpha_sb")
        nc.gpsimd.dma_start(alpha_sb, a_t.ap())
        # 2*alpha for the images binarized on DVE to +-0.5 (see below)
        alpha2_sb = consts.tile([128, 2], F32, name="alpha2_sb")
        nc.vector.tensor_scalar_mul(alpha2_sb, alpha_sb, 2.0)

        # rvA[p] = rv[p//32], rvB[p] = rv[4 + p//32] via selection matmuls
        rvAB = consts.tile([128, 2], F32, name="rvAB")
        for j, sel in enumerate((selA_sb, selB_sb)):
            ps_rv = pst.tile([128, 1], F32, name="ps_rv", tag="pst")
            nc.tensor.matmul(ps_rv, sel, rv8, start=True, stop=True)
            nc.vector.tensor_copy(rvAB[:, j : j + 1], ps_rv)

        # ---- weight inputs on the SP ring ahead of the x stream ----
        m_sb = zpool.tile([O_LOC, CT], F32, name="m_sb")
        nc.sync.dma_start(m_sb, m_t.ap())
        z_sb = zpool.tile([128, 2, CT], F32, name="z_sb")
        NQ = 4
        QW = CT // NQ
        for q in range(NQ):
            sl = slice(q * QW, (q + 1) * QW)
            nc.sync.dma_start(z_sb[:, :, sl], z_t.ap()[:, :, sl])

        xst = []
        for n in range(B_SH):
            xst.append(stage.tile([128, 2, H * H], F32, name=f"xst{n}", tag="xst"))
        for n in range(B_SH):
            xr = x_t.ap()[n].rearrange("(cc p) h w -> p cc (h w)", p=128)
            for cc in range(2):
                for hh in range(2):
                    sl = slice(hh * 392, (hh + 1) * 392)
                    nc.sync.dma_start(xst[n][:, cc, sl], xr[:, cc, sl])

        # ---- fold the k-pairs: t = rvA*z_b0 + rvB*z_b1, chunked to chase
        # the z DMA quarters ----
        t_sb = wpool.tile([128, CT], F32, name="t_sb")
        for q in range(NQ):
            sl = slice(q * QW, (q + 1) * QW)
            nc.vector.tensor_scalar_mul(t_sb[:, sl], z_sb[:, 1, sl], rvAB[:, 1:2])
            nc.vector.scalar_tensor_tensor(
                t_sb[:, sl],
                z_sb[:, 0, sl],
                rvAB[:, 0:1],
                t_sb[:, sl],
                op0=mybir.AluOpType.mult,
                op1=mybir.AluOpType.add,
            )

        # ---- fused a-reduce + transpose + M-add on the PE, sign on ACT.
        # Block idx2 = cc*9 + t -> wt_local cols [idx2*32, idx2*32+32).
        # Groups of 4 blocks share a [128, 128] psum, signed in one ACT op.
        wt_local = consts.tile([128, WCOLS], FP8, name="wt_local")
        wt_sign = None
        n_grp = (2 * KK + 3) // 4  # 5 groups (last has 2 blocks)
        for g in range(n_grp):
            blocks = [i for i in range(g * 4, min((g + 1) * 4, 2 * KK))]
            ps_blk = pst.tile([128, 32 * len(blocks)], F32, name="ps_blk", tag="pst")
            for bi, idx2 in enumerate(blocks):
                cc, t = divmod(idx2, KK)
                tb = bass.AP(
                    tensor=t_sb.tensor,
                    offset=t_sb.offset + cc * 128 * KK + t,
                    ap=[t_sb.ap[0], [KK, 128]],
                )
                mb = bass.AP(
                    tensor=m_sb.tensor,
                    offset=m_sb.offset + cc * 128 * KK + t,
                    ap=[m_sb.ap[0], [KK, 128]],
                )
                out_sl = ps_blk[:, bi * 32 : (bi + 1) * 32]
                nc.tensor.matmul(out_sl, tb, ssel_sb, start=True, stop=False)
                nc.tensor.matmul(out_sl, mb, eye_sb, start=False, stop=True)
            wt_sign = nc.scalar.sign(
                wt_local[:, g * 128 : g * 128 + 32 * len(blocks)], ps_blk
            )

        # ---- AllGather the fp8 weight slices (gpsimd ring + ncfw) ----
        nc.gpsimd.dma_start(ag_in.ap(), wt_local)
        nc.gpsimd.collective_compute(
            "AllGather",
            mybir.AluOpType.bypass,
            replica_groups=[list(range(N_CORES))],
            ins=[ag_in.ap()],
            outs=[ag_out.ap()],
        )
        # wt [128 c_low, 2 cc, 9 t, 256 o]; rank r owns o in [32r, 32r+32)
        wt = consts.tile([128, 2, KK, O], FP8, name="wt")
        for r in range(N_CORES):
            src = ag_out.ap()[r * 128 : (r + 1) * 128, :].rearrange(
                "p (e o) -> p e o", o=O_LOC
            )
            nc.gpsimd.dma_start(wt[:, :, :, r * O_LOC : (r + 1) * O_LOC], src)

        # ---- activations: zero padding borders (dep-free DVE), sign(x) ----
        act_tiles = []
        for n in range(B_SH):
            a_n = acts.tile([128, 2, PADW], FP8, name=f"a{n}", tag=f"a{n}")
            nc.vector.memset(a_n[:, :, 0:30], 0.0)
            nc.vector.memset(a_n[:, :, 870:PADW], 0.0)
            pairs = a_n[:, :, 29 : 29 + 29 * HP].rearrange(
                "p cc (r two) -> p cc r two", two=HP
            )[:, :, :, :2]
            nc.vector.memset(pairs, 0.0)
            interior = a_n[:, :, 31 : 31 + 28 * HP].rearrange(
                "p cc (r xx) -> p cc r xx", xx=HP
            )[:, :, :, :28]
            xv = xst[n].rearrange("p cc (h w) -> p cc h w", w=28)
            if n < 2:
                # first two images binarize on DVE to (x>=0)-0.5 = +-0.5 —
                # exact in fp8; their drains use 2*alpha — keeping the
                # in-order ACT queue free for the critical weight signs
                nc.vector.tensor_scalar(
                    interior,
                    xv,
                    0.0,
                    0.5,
                    op0=mybir.AluOpType.is_ge,
                    op1=mybir.AluOpType.subtract,
                )
            else:
                si = nc.scalar.sign(interior, xv)
                add_dep_helper(si.ins, wt_sign.ins, reason="wt signs gate x signs")
            act_tiles.append(a_n)

        # ---- conv: per image, 2 o-halves x 2 half-images; the two half-
        # image matmuls share each (tap, o-half) DoubleRow LDWEIGHTS ----
        for n in range(B_SH):
            a_n = act_tiles[n]
            for h in range(2):
                ps0 = psums.tile([128, 420], F32, name="ps0", tag="ps")
                ps1 = psums.tile([128, 420], F32, name="ps1", tag="ps")
                pss = (ps0, ps1)
                lhs = wt[:, :, :, h * 128 : (h + 1) * 128]
                for t in range(KK):
                    dy, dx = divmod(t, 3)
                    for half in range(2):
                        off = (half * 14 + dy) * HP + dx
                        nc.tensor.matmul(
                            pss[half],
                            lhs[:, :, t, :],
                            a_n[:, :, off : off + 420],
                            start=(t == 0),
                            stop=(t == KK - 1),
                            perf_mode=mybir.MatmulPerfMode.DoubleRow,
                        )
                a_col = alpha2_sb if n < 2 else alpha_sb
                for half in range(2):
                    ob = outp.tile([128, 392], F32, name="ob", tag="ob")
                    ps_v = pss[half].rearrange("p (r xx) -> p r xx", xx=HP)[:, :, :28]
                    ob_v = ob.rearrange("p (r xx) -> p r xx", xx=28)
                    # drains on DVE so the in-order ACT sign stream can't
                    # stall the psum recycle
                    nc.vector.tensor_scalar_mul(ob_v, ps_v, a_col[:, h : h + 1])
                    dst = out_t.ap()[n].rearrange("o h w -> o (h w)")[
                        h * 128 : (h + 1) * 128, half * 392 : (half + 1) * 392
                    ]
                    # out-writes ride the ACT HWDGE ring
                    nc.scalar.dma_start(dst, ob)


_PROGRAM = None


def build_program():
    global _PROGRAM
    if _PROGRAM is not None:
        return _PROGRAM
    nc = bacc.Bacc(
        "TRN2",
        target_bir_lowering=False,
        debug=False,
        enable_asserts=False,
        num_devices=N_CORES,
    )
    x_t = nc.dram_tensor("x", [B_SH, C, H, H], F32, kind="ExternalInput")
    z_t = nc.dram_tensor("Zp", [128, 2, CT], F32, kind="ExternalInput")
    m_t = nc.dram_tensor("Mp", [O_LOC, CT], F32, kind="ExternalInput")
    a_t = nc.dram_tensor("alphap", [128, 2], F32, kind="ExternalInput")
    rv_t = nc.dram_tensor("rv", [1, K], F32, kind="ExternalInput")
    selA, selB, s_sel, eye32 = _sel_consts()
    selA_t = nc.inline_tensor(selA, name="selA")
    selB_t = nc.inline_tensor(selB, name="selB")
    ssel_t = nc.inline_tensor(s_sel, name="ssel")
    eye_t = nc.inline_tensor(eye32, name="eye32")
    ag_in = nc.dram_tensor("ag_in", [128, WCOLS], FP8, kind="Internal")
    ag_out = nc.dram_tensor(
        "ag_out", [N_CORES * 128, WCOLS], FP8, kind="Internal", addr_space="Shared"
    )
    out_t = nc.dram_tensor("out", [B_SH, O, H, H], F32, kind="ExternalOutput")

    with tile.TileContext(nc) as tc:
        _build_kernel(tc, x_t, z_t, m_t, a_t, rv_t, selA_t, selB_t, ssel_t,
                      eye_t, ag_in, ag_out, out_t)
    nc.compile()
    _PROGRAM = nc
    return nc


def make_in_maps(x, M, Z, alpha, rv):
    x = np.ascontiguousarray(np.asarray(x, dtype=np.float32))
    M = np.asarray(M, dtype=np.float32).reshape(O, CT)
    Z = np.asarray(Z, dtype=np.float32).reshape(K, O, CT)
    alpha = np.asarray(alpha, dtype=np.float32).reshape(O)
    rv = np.ascontiguousarray(np.asarray(rv, dtype=np.float32))
    alphap = np.ascontiguousarray(alpha.reshape(2, 128).T)
    in_maps = []
    for i in range(N_CORES):
        osl = slice(i * O_LOC, (i + 1) * O_LOC)
        # Zp[a*32+oo, b, ct] = Z[b*4+a, oo, ct]
        zp = np.ascontiguousarray(
            Z[:, osl].reshape(2, 4, O_LOC, CT).transpose(1, 2, 0, 3).reshape(128, 2, CT)
        )
        in_maps.append(
            {
                "x": np.ascontiguousarray(x[i * B_SH : (i + 1) * B_SH]),
                "Zp": zp,
                "Mp": np.ascontiguousarray(M[osl]),
                "alphap": alphap,
                "rv": rv,
            }
        )
    return in_maps


def assemble_out(results):
    out = np.empty((B, O, H, H), dtype=np.float32)
    for i in range(N_CORES):
        r = np.asarray(results[i]["out"]).reshape(B_SH, O, H, H)
        out[i * B_SH : (i + 1) * B_SH] = r
    return out


def kernel(x, M, Z, alpha, rv, trace=False):
    nc = build_program()
    in_maps = make_in_maps(x, M, Z, alpha, rv)
    res = run_bass_kernel_spmd(
        nc, in_maps, core_ids=list(range(N_CORES)), trace=trace
    )
    if trace:
        kernel.last_results = res
    return assemble_out(res.results)


if __name__ == "__main__":
    build_program()
    print("program built ok")
